# revision 1
# baseline (speedup 1.0000x reference)
"""CenterLoss Trainium2 kernel (8 NeuronCores, data-parallel over batch).

Math: the reference builds the full [N, C] masked distance matrix, but only
the labeled entry of each row survives the mask, so

    loss = ( sum_i ||x_i - centers[labels_i]||^2  +  N*(C-1)*CLAMP_MIN ) / N

(the second term is the clamp applied to the zeroed-out entries). Per core:
gather centers rows via indirect DMA, DVE subtract, ACT square with row-
accumulate, PE ones-matmul for the cross-partition reduction. Host sums the
8 per-core partials.
"""

import numpy as np

import concourse.bacc as bacc
import concourse.tile as tile
from concourse import bass, mybir
from concourse.bass_utils import run_bass_kernel_spmd

N, C, D = 16384, 1024, 128
N_CORES = 8
NS = N // N_CORES  # 2048 rows per core
P = 128
T = NS // P  # 16 tiles of [128, 128] per core
CLAMP_MIN = 1e-12

_cache = {}


def build_nc():
    nc = bacc.Bacc()
    x = nc.declare_dram_parameter("x", [NS, D], mybir.dt.float32, isOutput=False)
    centers = nc.declare_dram_parameter(
        "centers", [C, D], mybir.dt.float32, isOutput=False
    )
    labels = nc.declare_dram_parameter("labels", [NS], mybir.dt.int32, isOutput=False)
    out = nc.declare_dram_parameter("out", [1, 1], mybir.dt.float32, isOutput=True)

    with tile.TileContext(nc) as tc:
        with (
            tc.tile_pool(name="sbuf", bufs=4) as pool,
            tc.tile_pool(name="acc", bufs=1) as accp,
            tc.tile_pool(name="psum", bufs=1, space="PSUM") as psump,
        ):
            acc = accp.tile([P, T], mybir.dt.float32)
            ones = accp.tile([P, 1], mybir.dt.float32)
            nc.vector.memset(ones[:], 1.0)
            for t in range(T):
                sl = slice(t * P, (t + 1) * P)
                x_tile = pool.tile([P, D], mybir.dt.float32)
                i_tile = pool.tile([P, 1], mybir.dt.int32)
                g_tile = pool.tile([P, D], mybir.dt.float32)
                d_tile = pool.tile([P, D], mybir.dt.float32)
                nc.sync.dma_start(out=x_tile[:], in_=x[sl, :])
                nc.sync.dma_start(out=i_tile[:], in_=labels[sl, None])
                nc.gpsimd.indirect_dma_start(
                    out=g_tile[:],
                    out_offset=None,
                    in_=centers[:],
                    in_offset=bass.IndirectOffsetOnAxis(ap=i_tile[:, :1], axis=0),
                )
                nc.vector.tensor_tensor(
                    out=d_tile[:],
                    in0=x_tile[:],
                    in1=g_tile[:],
                    op=mybir.AluOpType.subtract,
                )
                nc.scalar.activation(
                    out=d_tile[:],
                    in_=d_tile[:],
                    func=mybir.ActivationFunctionType.Square,
                    accum_out=acc[:, t : t + 1],
                )
            psum = psump.tile([1, T], mybir.dt.float32)
            nc.tensor.matmul(
                out=psum[:, :], lhsT=ones[:], rhs=acc[:], start=True, stop=True
            )
            res = accp.tile([1, 1], mybir.dt.float32)
            nc.vector.reduce_sum(
                out=res[:1, :1], in_=psum[:1, :], axis=mybir.AxisListType.X
            )
            nc.sync.dma_start(out=out[:, :], in_=res[:1, :1])
    nc.compile()
    return nc


def _get_nc():
    if "nc" not in _cache:
        _cache["nc"] = build_nc()
    return _cache["nc"]


def make_in_maps(x, centers, labels):
    x = np.ascontiguousarray(x, dtype=np.float32)
    centers = np.ascontiguousarray(centers, dtype=np.float32)
    labels32 = np.ascontiguousarray(labels.astype(np.int32))
    in_maps = []
    for c in range(N_CORES):
        sl = slice(c * NS, (c + 1) * NS)
        in_maps.append(
            {"x": x[sl], "centers": centers, "labels": labels32[sl]}
        )
    return in_maps


def finalize(results):
    total = sum(float(results[c]["out"][0, 0]) for c in range(N_CORES))
    total += N * (C - 1) * CLAMP_MIN
    return np.float32(total / N)


def kernel(x, centers, labels):
    nc = _get_nc()
    res = run_bass_kernel_spmd(
        nc, make_in_maps(x, centers, labels), core_ids=list(range(N_CORES))
    )
    return finalize(res.results)


# revision 2
# speedup vs baseline: 1.0066x; 1.0066x over previous
"""CenterLoss Trainium2 kernel (8 NeuronCores, data-parallel over batch).

Math: the reference builds the full [N, C] masked distance matrix, but only
the labeled entry of each row survives the mask, so

    loss = ( sum_i ||x_i - centers[labels_i]||^2  +  N*(C-1)*CLAMP_MIN ) / N

(the second term is the clamp applied to the zeroed-out entries).

Per core (2048 samples): one batched dma_gather pulls centers[labels] from
HBM directly into sample-major SBUF layout, DVE subtracts x, ACT squares
with row-accumulate, a ones-matmul reduces across partitions. Host sums the
8 per-core partials.
"""

import numpy as np

import concourse.bacc as bacc
import concourse.tile as tile
from concourse import bass, mybir
from concourse.bass_utils import run_bass_kernel_spmd

N, C, D = 16384, 1024, 128
N_CORES = 8
NS = N // N_CORES  # 2048 rows per core
P = 128
T = NS // P  # 16 tiles of [128, 128] per core
CLAMP_MIN = 1e-12

_cache = {}


def build_nc(n_chunk=4, act_split=True):
    """n_chunk: pipeline chunks over the 16 sample tiles (must divide T)."""
    assert T % n_chunk == 0
    tpc = T // n_chunk  # tiles per chunk

    nc = bacc.Bacc()
    x = nc.declare_dram_parameter("x", [NS, D], mybir.dt.float32, isOutput=False)
    centers = nc.declare_dram_parameter(
        "centers", [C, D], mybir.dt.float32, isOutput=False
    )
    labels = nc.declare_dram_parameter(
        "labels", [P, NS // 16], mybir.dt.int16, isOutput=False
    )
    out = nc.declare_dram_parameter("out", [1, 1], mybir.dt.float32, isOutput=True)

    x_t = x.rearrange("(t p) d -> p t d", p=P)  # [128, T, 128]

    with tile.TileContext(nc) as tc:
        with (
            tc.tile_pool(name="data", bufs=1) as data,
            tc.tile_pool(name="small", bufs=1) as small,
            tc.tile_pool(name="psum", bufs=1, space="PSUM") as psump,
        ):
            x_sb = data.tile([P, T, D], mybir.dt.float32)
            g_sb = data.tile([P, T, D], mybir.dt.float32)
            d_sb = data.tile([P, T, D], mybir.dt.float32)
            i_sb = small.tile([P, NS // 16], mybir.dt.int16)
            acc = small.tile([P, n_chunk], mybir.dt.float32)
            ones = small.tile([P, 1], mybir.dt.float32)

            nc.vector.memset(ones[:], 1.0)
            nc.sync.dma_start(out=i_sb[:], in_=labels[:, :])
            for k in range(n_chunk):
                ts = slice(k * tpc, (k + 1) * tpc)
                nc.sync.dma_start(out=x_sb[:, ts, :], in_=x_t[:, ts, :])
                # gather centers[labels] for this chunk's samples
                nidx = tpc * P
                nc.gpsimd.dma_gather(
                    out_ap=g_sb[:, ts, :],
                    in_ap=centers[:],
                    idxs_ap=i_sb[:, k * (nidx // 16) : (k + 1) * (nidx // 16)],
                    num_idxs=nidx,
                    num_idxs_reg=nidx,
                    elem_size=D,
                )
                nc.vector.tensor_tensor(
                    out=d_sb[:, ts, :],
                    in0=x_sb[:, ts, :],
                    in1=g_sb[:, ts, :],
                    op=mybir.AluOpType.subtract,
                )
                if act_split:
                    # square + row-accumulate on the scalar engine
                    nc.scalar.activation(
                        out=d_sb[:, ts, :],
                        in_=d_sb[:, ts, :],
                        func=mybir.ActivationFunctionType.Square,
                        accum_out=acc[:, k : k + 1],
                    )
                else:
                    nc.vector.tensor_tensor_reduce(
                        out=d_sb[:, ts, :],
                        in0=d_sb[:, ts, :],
                        in1=d_sb[:, ts, :],
                        scale=1.0,
                        scalar=0.0,
                        op0=mybir.AluOpType.mult,
                        op1=mybir.AluOpType.add,
                        accum_out=acc[:, k : k + 1],
                    )
            psum = psump.tile([1, n_chunk], mybir.dt.float32)
            nc.tensor.matmul(
                out=psum[:, :], lhsT=ones[:], rhs=acc[:], start=True, stop=True
            )
            res = small.tile([1, 1], mybir.dt.float32)
            nc.vector.reduce_sum(
                out=res[:1, :1], in_=psum[:1, :], axis=mybir.AxisListType.X
            )
            nc.sync.dma_start(out=out[:, :], in_=res[:1, :1])
    nc.compile()
    return nc


def _get_nc():
    if "nc" not in _cache:
        _cache["nc"] = build_nc()
    return _cache["nc"]


def wrap_labels(labels_shard):
    """[NS] int -> [128, NS//16] int16 wrapped per 16-partition group,
    replicated across the 8 gpsimd cores."""
    w = labels_shard.reshape(NS // 16, 16).T.astype(np.int16)  # [16, NS//16]
    return np.ascontiguousarray(np.tile(w, (8, 1)))


def make_in_maps(x, centers, labels):
    x = np.ascontiguousarray(x, dtype=np.float32)
    centers = np.ascontiguousarray(centers, dtype=np.float32)
    labels = np.asarray(labels)
    in_maps = []
    for c in range(N_CORES):
        sl = slice(c * NS, (c + 1) * NS)
        in_maps.append(
            {
                "x": x[sl],
                "centers": centers,
                "labels": wrap_labels(labels[sl]),
            }
        )
    return in_maps


def finalize(results):
    total = sum(float(results[c]["out"][0, 0]) for c in range(N_CORES))
    total += N * (C - 1) * CLAMP_MIN
    return np.float32(total / N)


def kernel(x, centers, labels):
    nc = _get_nc()
    res = run_bass_kernel_spmd(
        nc, make_in_maps(x, centers, labels), core_ids=list(range(N_CORES))
    )
    return finalize(res.results)
